# revision 2
# baseline (speedup 1.0000x reference)
"""GCN regression kernel for Trainium2, 8 NeuronCores (SPMD).

Sharding: nodes range-sharded across 8 cores (6250 each, padded to 6272 =
49 windows of 128).  Per layer each core computes g = dinv*(h @ W) for its
own nodes, an AllGather assembles the full 50176-row g-table, a chunked
SWDGE dma_gather pulls g[src] rows for the core's (dst-sorted, window/
src-half bucketed) edges, and PE matmuls with 0/1 indicator matrices
(built on DVE via iota/is_equal) accumulate agg[dst_local, f] per 128-node
window in PSUM.  Eviction fuses self-loop, dinv scaling, bias, ReLU.
Pooling = indicator matmuls over graph ids + lin_w dot; host combines
per-core partials.
"""
import sys
sys.path.insert(0, "/opt/trn_rl_repo")
import time
import numpy as np

N = 50000
E = 800000
D = 128
G = 512
L = 3
P = 8
NS = N // P          # 6250 owned nodes per core
NW = 49              # windows of 128 (6272 padded)
NSP = NW * 128       # 6272
HALF = NSP * P // 2  # 25088 table rows per src half

_cache = {}


def _wrap_idx_cols(idx):
    """Pack indices j -> [j%16, j//16] in 16 rows, replicated to 128 partitions."""
    n = len(idx)
    assert n % 128 == 0
    a = np.zeros((16, n // 16), np.int16)
    a[np.arange(n) % 16, np.arange(n) // 16] = idx.astype(np.int16)
    return np.tile(a, (8, 1))


def _col_layout(vals, fill):
    """[NW*128] values -> [128, NW] with [p, w] = vals[w*128 + p]."""
    arr = np.full(NW * 128, fill, np.float32)
    arr[: len(vals)] = vals
    return np.ascontiguousarray(arr.reshape(NW, 128).T)


def preprocess(edge_index, batch):
    src = np.asarray(edge_index[0], np.int64)
    dst = np.asarray(edge_index[1], np.int64)
    batch = np.asarray(batch, np.int64)
    deg = np.bincount(dst, minlength=N).astype(np.float64) + 1.0
    dinv = (1.0 / np.sqrt(deg)).astype(np.float32)

    trow = (src // NS) * NSP + (src % NS)  # table row (padded shards)
    half = trow // HALF

    # bucket edges per core / window / half
    buckets = [[[None, None] for _ in range(NW)] for _ in range(P)]
    for c in range(P):
        base = c * NS
        sel = (dst // NS) == c
        s_tr, s_half = trow[sel], half[sel]
        s_dl = dst[sel] - base
        s_w = s_dl // 128
        for h in range(2):
            m_h = s_half == h
            w_h, tr_h, dl_h = s_w[m_h], s_tr[m_h], s_dl[m_h]
            order = np.argsort(w_h, kind="stable")
            w_h, tr_h, dl_h = w_h[order], tr_h[order], dl_h[order]
            splits = np.searchsorted(w_h, np.arange(NW + 1))
            for w in range(NW):
                lo, hi = splits[w], splits[w + 1]
                buckets[c][w][h] = (tr_h[lo:hi] - h * HALF, dl_h[lo:hi] - 128 * w)

    # common call structure: per (w, h) block count = max over cores
    call_specs = []  # (w, h, nb, blk_off, col_off)
    blk_off = col_off = 0
    for w in range(NW):
        for h in range(2):
            mx = max(len(buckets[c][w][h][0]) for c in range(P))
            if mx == 0:
                continue
            nb = -(-mx // 128)
            call_specs.append((w, h, nb, blk_off, col_off))
            blk_off += nb
            col_off += nb * 8
    totblk, idxcols = blk_off, col_off

    cores = []
    for c in range(P):
        idx_arr = np.zeros((128, idxcols), np.int16)
        dstrel = np.full((128, totblk), -1.0, np.float32)
        for (w, h, nb, boff, coff) in call_specs:
            tr, dl = buckets[c][w][h]
            cnt = len(tr)
            npad = nb * 128
            idxs = np.zeros(npad, np.int64)
            idxs[:cnt] = tr
            idx_arr[:, coff : coff + nb * 8] = _wrap_idx_cols(idxs)
            dr = np.full(npad, -1.0, np.float32)
            dr[:cnt] = dl.astype(np.float32)
            dstrel[:, boff : boff + nb] = dr.reshape(nb, 128).T

        base = c * NS
        g0 = int(batch[base])
        gspan = int(batch[base + NS - 1]) - g0 + 1
        assert gspan <= 128, f"core {c}: graph span {gspan} > 128"
        cores.append(
            dict(
                idx_arr=idx_arr,
                dstrel=dstrel,
                dinvc=_col_layout(dinv[base : base + NS], 0.0),
                brel=_col_layout((batch[base : base + NS] - g0).astype(np.float32), -1.0),
                g0=g0,
                gspan=gspan,
            )
        )

    counts = np.bincount(batch, minlength=G).astype(np.float32)
    return cores, counts, call_specs, totblk, idxcols


def build(call_specs, totblk, idxcols):
    from concourse import bacc, tile, mybir

    f32 = mybir.dt.float32
    i16 = mybir.dt.int16
    nbmax = max(nb for (_, _, nb, _, _) in call_specs)

    nc = bacc.Bacc("TRN2", target_bir_lowering=False, debug=False, num_devices=P)
    x_in = nc.dram_tensor("x_own", [NSP, D], f32, kind="ExternalInput")
    Ws_in = nc.dram_tensor("Ws", [L * D, D], f32, kind="ExternalInput")
    brep_in = nc.dram_tensor("brep", [L * 128, D], f32, kind="ExternalInput")
    lin_in = nc.dram_tensor("linrep", [128, D], f32, kind="ExternalInput")
    ident_in = nc.dram_tensor("ident", [128, 128], f32, kind="ExternalInput")
    iota_in = nc.dram_tensor("iota", [128, 128], f32, kind="ExternalInput")
    idx_in = nc.dram_tensor("idxs", [128, idxcols], i16, kind="ExternalInput")
    dstrel_in = nc.dram_tensor("dstrel", [128, totblk], f32, kind="ExternalInput")
    dinvc_in = nc.dram_tensor("dinvc", [128, NW], f32, kind="ExternalInput")
    brel_in = nc.dram_tensor("brel", [128, NW], f32, kind="ExternalInput")
    gsel_in = nc.dram_tensor("gsel", [128, 1], f32, kind="ExternalInput")
    out_d = nc.dram_tensor("out_partial", [G, 1], f32, kind="ExternalOutput")

    with tile.TileContext(nc) as tc:
        with (
            tc.tile_pool(name="dram", bufs=1, space="DRAM") as dram,
            tc.tile_pool(name="const", bufs=1) as cp,
            tc.tile_pool(name="state", bufs=1) as st,
            tc.tile_pool(name="gt", bufs=6) as gtp,
            tc.tile_pool(name="work", bufs=4) as wp,
            tc.tile_pool(name="ps", bufs=2, space="PSUM") as psA,
            tc.tile_pool(name="psg", bufs=2, space="PSUM") as psG,
            tc.tile_pool(name="agg", bufs=3, space="PSUM") as psAgg,
            tc.tile_pool(name="pool", bufs=1, space="PSUM") as psPool,
        ):
            W_sb = cp.tile([128, L * D], f32)
            brep_sb = cp.tile([128, L * D], f32)
            for l in range(L):
                nc.sync.dma_start(
                    W_sb[:, l * D : (l + 1) * D], Ws_in[l * D : (l + 1) * D, :]
                )
                nc.sync.dma_start(
                    brep_sb[:, l * D : (l + 1) * D], brep_in[l * 128 : (l + 1) * 128, :]
                )
            lin_sb = cp.tile([128, D], f32)
            nc.sync.dma_start(lin_sb[:], lin_in[:])
            ident = cp.tile([128, 128], f32)
            nc.sync.dma_start(ident[:], ident_in[:])
            iota = cp.tile([128, 128], f32)
            nc.sync.dma_start(iota[:], iota_in[:])
            idx_sb = cp.tile([128, idxcols], i16)
            nc.sync.dma_start(idx_sb[:], idx_in[:])
            dstrel_sb = cp.tile([128, totblk], f32)
            nc.sync.dma_start(dstrel_sb[:], dstrel_in[:])
            dinv_sb = cp.tile([128, NW], f32)
            nc.sync.dma_start(dinv_sb[:], dinvc_in[:])
            brel_sb = cp.tile([128, NW], f32)
            nc.sync.dma_start(brel_sb[:], brel_in[:])
            gsel_sb = cp.tile([128, 1], f32)
            nc.sync.dma_start(gsel_sb[:], gsel_in[:])

            hbuf = st.tile([128, NW, D], f32)
            gbuf = st.tile([128, NW, D], f32)
            for w in range(NW):
                nc.sync.dma_start(hbuf[:, w, :], x_in[w * 128 : (w + 1) * 128, :])

            bounce = dram.tile([NSP, D], f32)
            tables = [
                dram.tile([P * NSP, D], f32, addr_space="Shared",
                          name=f"tbl{l}", tag=f"tbl{l}")
                for l in range(L)
            ]

            for l in range(L):
                # ---- own g slice ----
                for w in range(NW):
                    tp = psA.tile([128, 128], f32, name="tp", tag="tp")
                    nc.tensor.transpose(tp[:], hbuf[:, w, :], ident[:])
                    hT = wp.tile([128, 128], f32, name="hT", tag="hT")
                    nc.vector.tensor_copy(hT[:], tp[:])
                    gp = psG.tile([128, 128], f32, name="gp", tag="gp")
                    nc.tensor.matmul(
                        gp[:], hT[:], W_sb[:, l * D : (l + 1) * D],
                        start=True, stop=True,
                    )
                    nc.vector.tensor_scalar(
                        gbuf[:, w, :], gp[:], dinv_sb[:, w : w + 1], None,
                        op0=mybir.AluOpType.mult,
                    )
                    nc.sync.dma_start(
                        bounce[w * 128 : (w + 1) * 128, :], gbuf[:, w, :]
                    )
                nc.gpsimd.collective_compute(
                    "AllGather",
                    mybir.AluOpType.bypass,
                    replica_groups=[list(range(P))],
                    ins=[bounce[:]],
                    outs=[tables[l][:]],
                )

                # ---- gather + aggregate ----
                agg_by_w = {}
                for (w, h, nb, blk_off, col_off) in call_specs:
                    gt = gtp.tile([128, nbmax, 128], f32, name="gt", tag="gt")
                    nc.gpsimd.dma_gather(
                        gt[:, :nb, :],
                        tables[l][h * HALF : (h + 1) * HALF, :],
                        idx_sb[:, col_off : col_off + nb * 8],
                        nb * 128, nb * 128, 128,
                        single_packet=False,
                    )
                    agg_by_w.setdefault(w, []).append((gt, nb, blk_off))
                for w in range(NW):
                    entries = agg_by_w.get(w, [])
                    nmm = sum(nb for (_, nb, _) in entries)
                    ap = psAgg.tile([128, 128], f32, name="ag", tag="ag")
                    mi = 0
                    for (gt, nb, blk_off) in entries:
                        for b in range(nb):
                            ind = wp.tile([128, 128], f32, name="ind", tag="ind")
                            nc.vector.tensor_scalar(
                                ind[:], iota[:],
                                dstrel_sb[:, blk_off + b : blk_off + b + 1], None,
                                op0=mybir.AluOpType.is_equal,
                            )
                            nc.tensor.matmul(
                                ap[:], ind[:], gt[:, b, :],
                                start=(mi == 0), stop=(mi == nmm - 1),
                            )
                            mi += 1
                    # ---- evict: h = relu(dinv*(agg + g) + b) ----
                    s1 = wp.tile([128, 128], f32, name="s1", tag="s1")
                    if nmm > 0:
                        nc.vector.tensor_tensor(
                            s1[:], ap[:], gbuf[:, w, :], op=mybir.AluOpType.add
                        )
                    else:
                        nc.vector.tensor_copy(s1[:], gbuf[:, w, :])
                    s2 = wp.tile([128, 128], f32, name="s2", tag="s2")
                    nc.vector.tensor_scalar(
                        s2[:], s1[:], dinv_sb[:, w : w + 1], None,
                        op0=mybir.AluOpType.mult,
                    )
                    s3 = wp.tile([128, 128], f32, name="s3", tag="s3")
                    nc.vector.tensor_tensor(
                        s3[:], s2[:], brep_sb[:, l * D : (l + 1) * D],
                        op=mybir.AluOpType.add,
                    )
                    nc.vector.tensor_scalar(
                        hbuf[:, w, :], s3[:], 0.0, None, op0=mybir.AluOpType.max
                    )

            # ---- pooling ----
            pp = psPool.tile([128, 128], f32)
            for w in range(NW):
                pol = wp.tile([128, 128], f32, name="pol", tag="ind")
                nc.vector.tensor_scalar(
                    pol[:], iota[:], brel_sb[:, w : w + 1], None,
                    op0=mybir.AluOpType.is_equal,
                )
                nc.tensor.matmul(
                    pp[:], pol[:], hbuf[:, w, :], start=(w == 0), stop=(w == NW - 1)
                )
            pl = wp.tile([128, 128], f32, name="pl", tag="s1")
            nc.vector.tensor_tensor(pl[:], pp[:], lin_sb[:], op=mybir.AluOpType.mult)
            red = wp.tile([128, 1], f32, name="red", tag="red")
            nc.vector.tensor_reduce(
                red[:], pl[:], mybir.AxisListType.X, mybir.AluOpType.add
            )
            redm = wp.tile([128, 1], f32, name="redm", tag="red2")
            nc.vector.tensor_tensor(
                redm[:], red[:], gsel_sb[:], op=mybir.AluOpType.mult
            )
            nc.sync.dma_start(out_d[0:128, :], redm[:])
    nc.compile()
    return nc


class _Runner:
    def __init__(self, nc, n_cores):
        import jax
        from jax.sharding import Mesh, PartitionSpec
        from jax.experimental.shard_map import shard_map
        from concourse import mybir
        from concourse.bass2jax import (
            _bass_exec_p, install_neuronx_cc_hook, partition_id_tensor,
        )

        install_neuronx_cc_hook()
        self.n_cores = n_cores
        partition_name = nc.partition_id_tensor.name if nc.partition_id_tensor else None
        in_names, out_names, out_avals, zero_outs = [], [], [], []
        for alloc in nc.m.functions[0].allocations:
            if not isinstance(alloc, mybir.MemoryLocationSet):
                continue
            name = alloc.memorylocations[0].name
            if alloc.kind == "ExternalInput":
                if name != partition_name:
                    in_names.append(name)
            elif alloc.kind == "ExternalOutput":
                out_names.append(name)
                out_avals.append(
                    jax.core.ShapedArray(
                        tuple(alloc.tensor_shape), mybir.dt.np(alloc.dtype)
                    )
                )
                zero_outs.append(
                    np.zeros(tuple(alloc.tensor_shape), mybir.dt.np(alloc.dtype))
                )
        self.n_params = len(in_names)
        self.in_names = in_names + out_names + (
            [partition_name] if partition_name else []
        )
        self.out_names = out_names
        self.out_avals = out_avals
        self.zero_outs = zero_outs
        donate = tuple(range(self.n_params, self.n_params + len(out_names)))
        out_avals_t, in_names_t, out_names_t = (
            tuple(out_avals), tuple(self.in_names), tuple(out_names),
        )

        def _body(*args):
            operands = list(args)
            if partition_name is not None:
                operands.append(partition_id_tensor())
            return tuple(
                _bass_exec_p.bind(
                    *operands,
                    out_avals=out_avals_t,
                    in_names=in_names_t,
                    out_names=out_names_t,
                    lowering_input_output_aliases=(),
                    sim_require_finite=True,
                    sim_require_nnan=True,
                    nc=nc,
                )
            )

        devices = jax.devices()[:n_cores]
        mesh = Mesh(np.asarray(devices), ("core",))
        n_io = self.n_params + len(out_names)
        self.fn = jax.jit(
            shard_map(
                _body, mesh=mesh,
                in_specs=(PartitionSpec("core"),) * n_io,
                out_specs=(PartitionSpec("core"),) * len(out_names),
                check_rep=False,
            ),
            donate_argnums=donate,
            keep_unused=True,
        )

    def __call__(self, in_maps):
        n = self.n_cores
        per_core = [
            [np.asarray(m[name]) for name in self.in_names[: self.n_params]]
            for m in in_maps
        ]
        concat_in = [
            np.concatenate([per_core[c][i] for c in range(n)], axis=0)
            for i in range(self.n_params)
        ]
        concat_zeros = [
            np.zeros((n * z.shape[0], *z.shape[1:]), z.dtype) for z in self.zero_outs
        ]
        t0 = time.time()
        out_arrs = self.fn(*concat_in, *concat_zeros)
        out_arrs = [np.asarray(a) for a in out_arrs]
        wall = time.time() - t0
        results = [
            {
                name: out_arrs[i].reshape(n, *self.out_avals[i].shape)[c]
                for i, name in enumerate(self.out_names)
            }
            for c in range(n)
        ]
        return results, wall


def make_in_maps(x, Ws, bs, lin_w, cores):
    x = np.asarray(x, np.float32)
    Ws = np.asarray(Ws, np.float32).reshape(L * D, D)
    bs = np.asarray(bs, np.float32)
    lin_w = np.asarray(lin_w, np.float32)
    brep = np.concatenate([np.tile(bs[l][None, :], (128, 1)) for l in range(L)], 0)
    linrep = np.tile(lin_w.reshape(1, D), (128, 1)).astype(np.float32)
    ident = np.eye(128, dtype=np.float32)
    iota = np.tile(np.arange(128, dtype=np.float32)[None, :], (128, 1))
    in_maps = []
    for c in range(P):
        base = c * NS
        x_own = np.zeros((NSP, D), np.float32)
        x_own[:NS] = x[base : base + NS]
        core = cores[c]
        gsel = np.zeros((128, 1), np.float32)
        gsel[: core["gspan"]] = 1.0
        in_maps.append(
            {
                "x_own": x_own,
                "Ws": Ws,
                "brep": brep,
                "linrep": linrep,
                "ident": ident,
                "iota": iota,
                "idxs": core["idx_arr"],
                "dstrel": core["dstrel"],
                "dinvc": core["dinvc"],
                "brel": core["brel"],
                "gsel": gsel,
            }
        )
    return in_maps


def get_runner(edge_index, batch):
    key = (
        hash(np.asarray(edge_index).tobytes()),
        hash(np.asarray(batch).tobytes()),
    )
    if key in _cache:
        return _cache[key]
    cores, counts, call_specs, totblk, idxcols = preprocess(edge_index, batch)
    nc = build(call_specs, totblk, idxcols)
    runner = _Runner(nc, P)
    _cache[key] = (runner, cores, counts)
    return _cache[key]


def kernel(x, edge_index, batch, Ws, bs, lin_w, lin_b, _timing=None):
    runner, cores, counts = get_runner(edge_index, batch)
    in_maps = make_in_maps(x, Ws, bs, lin_w, cores)
    results, wall = runner(in_maps)
    if _timing is not None:
        _timing.append(wall)
    total = np.zeros(G, np.float64)
    for c in range(P):
        part = results[c]["out_partial"][:, 0].astype(np.float64)
        g0 = cores[c]["g0"]
        span = cores[c]["gspan"]
        hi = min(g0 + span, G)
        total[g0:hi] += part[: hi - g0]
    out = total / np.maximum(counts, 1.0) + float(np.asarray(lin_b).reshape(-1)[0])
    return out.astype(np.float32)
